# revision 17
# baseline (speedup 1.0000x reference)
"""CapsuleLayer dynamic-routing kernel for 8 TRN2 NeuronCores.

Problem: inputs [256,1152,8] f32, W [1152,10,8,16] f32, bias [1,1152,10,1] f32.
  u_hat = einsum('bid,icdv->bicv', inputs, W)
  3 rounds of routing (softmax over c, weighted sum over i, squash over v).
Output: [256, 10, 16] f32.

Sharding: 2-way batch x 4-way input-capsule (i) grid over 8 cores.
Core k: batch half k//4 (128 rows), i-quarter k%4 (288 i's).
Per-round partial sums over i are combined with an AllReduce over each
group of 4 cores ([0..3] and [4..7]). Output halves read from cores 0, 4.

v2 design: the weighted-sum stage s = sum_i c*u_hat runs on the PE
instead of DVE/Pool chains:
  - coupling weights cw[b,(c,j)] are transposed by PE identity matmuls
    into cwT[(c,j), b] tiles,
  - xcT[(c,d,j), b] = cwT * xT elementwise (DVE, 2x bf16),
  - s^T[(c,v), b] accumulates in PSUM via dense [96j,16v] W2 matmuls,
  - tiny [16,128] PE transposes bring s back to batch-partitioned form.
Logits l = sum_v u_hat*v remain mul+halving-tree chunks on DVE with some
chunks offloaded to Pool. exp on Act scatters (j,c)->(c,j) so softmax
tail ops stay in DVE 2x mode.
"""

import sys

if "/opt/trn_rl_repo" not in sys.path:
    sys.path.insert(0, "/opt/trn_rl_repo")

import numpy as np
import ml_dtypes

import concourse.bass as bass
from concourse import bacc, mybir, tile
from concourse.bass_utils import run_bass_kernel_spmd

F32 = mybir.dt.float32
BF16 = mybir.dt.bfloat16
AX = mybir.AxisListType
ALU = mybir.AluOpType
ACTF = mybir.ActivationFunctionType

B, I, D, C, V = 256, 1152, 8, 10, 16
CV = C * V                     # 160
NB = 128                       # batch rows per core
IQ = 288                       # i's per core ("j" below)
NG = IQ // 4                   # 72 groups of 4 i's (K=32 block-diag matmuls)
NT = NG // 3                   # 24 tiles of 96 partition-rows
EPS = 1e-7

# logits chunk plan: (j-start, width, on_pool); Pool keeps narrow chunks so
# its scratch stays small, DVE uses wide chunks to amortize per-op overheads
# logits chunks: 18 of width 16; the first four run on Pool
CHUNK_PLAN = [(16 * k, 16, k < 4) for k in range(18)]
Z_PLAN = [(48 * b, 48) for b in range(6)]
JT = 3                         # 96-row j-tiles per (c) or (d)

REPLICA_GROUPS = [[0, 1, 2, 3], [4, 5, 6, 7]]

# logits chunks offloaded to Pool (gpsimd), per round

# xc muls offloaded to Pool in the ws phase (empty: a slow Pool mul stalls
# the in-order PE queue and resets its pstate ramp)
POOL_XC = set()


def _ap(ap, dims):
    """Build an AP with explicit [step, count] free dims (partition dim kept)."""
    return bass.AP(ap.tensor, ap.offset, [list(ap.ap[0])] + [list(d) for d in dims])


def _squash(nc, pool, s_in, v_out):
    """v = (|s|^2/(1+|s|^2)) * s / sqrt(|s|^2 + EPS), norms over v (16).

    s_in: [128, 160] f32-ish SBUF AP. Writes v_out (bf16 for routing rounds,
    f32 for the final output round)."""
    sq = pool.tile([128, CV], F32, tag="sq")
    n2 = pool.tile([128, C], F32, tag="n2")
    n2e = pool.tile([128, C], F32, tag="n2e")
    qs = pool.tile([128, C], F32, tag="qs")
    mm = pool.tile([128, C], F32, tag="mm")
    rm = pool.tile([128, C], F32, tag="rm")
    fc = pool.tile([128, C], F32, tag="fc")
    nc.vector.tensor_mul(sq[:], s_in, s_in)
    nc.vector.tensor_reduce(
        n2[:], sq[:].rearrange("p (c v) -> p c v", v=V), axis=AX.X, op=ALU.add
    )
    # f = n2 / ((1+n2) * sqrt(n2+eps))
    nc.vector.tensor_scalar_add(n2e[:], n2[:], EPS)
    nc.scalar.activation(qs[:], n2e[:], ACTF.Sqrt)
    nc.vector.scalar_tensor_tensor(
        mm[:], n2[:], 1.0, qs[:], op0=ALU.add, op1=ALU.mult
    )
    nc.vector.reciprocal(rm[:], mm[:])
    nc.vector.tensor_mul(fc[:], n2[:], rm[:])
    # v = s * f (broadcast f over v)
    f_b = _ap(fc[:], [[1, C], [0, V]])
    s3 = s_in.rearrange("p (c v) -> p c v", v=V)
    nc.vector.tensor_mul(v_out[:].rearrange("p (c v) -> p c v", v=V), s3, f_b)


def _emit(nc, tc, use_bias, cc_stub=False):
    xt2_d = nc.declare_dram_parameter("xt2", [96, NT * 128], BF16, isOutput=False)
    wbd_d = nc.declare_dram_parameter("wbd", [96, NT * 640], BF16, isOutput=False)
    w2d_d = nc.declare_dram_parameter("w2d", [96, NT * CV], BF16, isOutput=False)
    xt_d = nc.declare_dram_parameter("xt", [96, 24 * 128], BF16, isOutput=False)
    w2_d = nc.declare_dram_parameter("w2", [96, 240 * 16], BF16, isOutput=False)
    id_d = nc.declare_dram_parameter("ident", [128, 128], BF16, isOutput=False)
    id32_d = nc.declare_dram_parameter("ident32", [128, 128], F32, isOutput=False)
    if use_bias:
        bias_d = nc.declare_dram_parameter("biasr", [128, IQ * C], BF16, isOutput=False)
    out_d = nc.declare_dram_parameter("out", [128, CV], F32, isOutput=True)

    WBC = 3                    # NT-tiles per streamed wbd chunk
    NWB = NT // WBC            # 8 chunks

    with (
        tc.tile_pool(name="const", bufs=1) as cp,
        tc.tile_pool(name="wbd_ring", bufs=2) as wp,
        tc.tile_pool(name="small", bufs=1) as sp,
        tc.tile_pool(name="ring", bufs=2) as rp,
        tc.tile_pool(name="gscr", bufs=1) as gp_scr,
        tc.tile_pool(name="xcr", bufs=3) as xp,
        tc.tile_pool(name="cwt", bufs=1) as cwp,
        tc.tile_pool(name="ps0", bufs=1, space="PSUM") as ps0p,
        tc.tile_pool(name="psg", bufs=3, space="PSUM") as psgp,
        tc.tile_pool(name="psT", bufs=1, space="PSUM") as psTp,
        tc.tile_pool(name="dram", bufs=1, space="DRAM") as dp,
    ):
        xt2 = cp.tile([96, NT * 128], BF16, tag="xt2")
        w2d = cp.tile([96, NT * CV], BF16, tag="w2d")
        xt = cp.tile([96, 24 * 128], BF16, tag="xt")
        w2 = cp.tile([96, 240 * 16], BF16, tag="w2")
        ident = cp.tile([128, 128], BF16, tag="ident")
        ident32 = cp.tile([128, 128], F32, tag="ident32")
        uhat = cp.tile([128, IQ * CV], BF16, tag="uhat")

        xsl = slice(0, 6 * 128)
        nc.sync.dma_start(xt2[:, xsl], xt2_d[:, xsl])
        wbd_tiles = []
        for wch in range(NWB):
            wt = wp.tile([96, WBC * 640], BF16, tag="wbd")
            if wch < 2:
                nc.sync.dma_start(
                    wt[:], wbd_d[:, wch * WBC * 640 : (wch + 1) * WBC * 640]
                )
            wbd_tiles.append(wt)
        for xch in range(1, 4):
            xsl = slice(xch * 6 * 128, (xch + 1) * 6 * 128)
            nc.sync.dma_start(xt2[:, xsl], xt2_d[:, xsl])
        nc.sync.dma_start(w2d[:], w2d_d[:])
        for wch in range(2, NWB):
            nc.sync.dma_start(
                wbd_tiles[wch][:],
                wbd_d[:, wch * WBC * 640 : (wch + 1) * WBC * 640],
            )
        nc.sync.dma_start(xt[:], xt_d[:])
        nc.sync.dma_start(w2[:], w2_d[:])
        nc.sync.dma_start(ident[:], id_d[:])
        nc.sync.dma_start(ident32[:], id32_d[:])
        if use_bias:
            biasr = cp.tile([128, IQ * C], BF16, tag="biasr")
            nc.sync.dma_start(biasr[:], bias_d[:])

        # persistent small tiles
        warm = sp.tile([128, 1], F32, tag="warm")
        nc.vector.memset(warm[:], 1.0)

        warm2 = sp.tile([128, 1], F32, tag="warm2")

        def prewarm(func, anchor=None):
            # dummy op so the ACT table set loads off the critical path;
            # anchor ties it to freshly-written data so the tile scheduler
            # cannot hoist it ahead of the ops it must follow
            if anchor is None:
                nc.scalar.activation(warm2[:], warm[:], func)
            else:
                # scale=0 keeps the data dependency but feeds func(0)
                nc.scalar.activation(warm2[:], anchor, func, scale=0.0)

        prewarm(ACTF.Sqrt)
        v_f = sp.tile([128, CV], F32, tag="v_f")
        v_b = sp.tile([128, CV], BF16, tag="v_b")
        s_part = sp.tile([128, CV], F32, tag="s_part")
        sTe = sp.tile([128, 3 * 128], BF16, tag="sTe")
        s_part32 = sp.tile([128, CV], F32, tag="s_part32")
        s_tot = sp.tile([128, CV], F32, tag="s_tot")
        ta = sp.tile([128, IQ * C], BF16, tag="ta")   # raw logits (j,c), rotates
        tb = sp.tile([128, IQ * C], BF16, tag="tb")
        et = sp.tile([128, IQ * C], BF16, tag="et")   # exp(logits), (c,j) order
        cw = sp.tile([128, IQ * C], BF16, tag="cw")   # softmax couplings (c,j)
        zsum = sp.tile([128, IQ], F32, tag="zsum")
        rz = sp.tile([128, IQ], BF16, tag="rz")

        def all_reduce(rnd, src, dst, dt, n):
            ccin = dp.tile([128, n], dt, tag=f"ccin{rnd}")
            ccout = dp.tile([128, n], dt, tag=f"ccout{rnd}")
            nc.scalar.dma_start(ccin[:], src[:, :n])
            if cc_stub:
                nc.scalar.dma_start(ccout[:], ccin[:])
            else:
                nc.gpsimd.collective_compute(
                    "AllReduce",
                    ALU.add,
                    replica_groups=REPLICA_GROUPS,
                    ins=[ccin.opt()],
                    outs=[ccout.opt()],
                )
            nc.scalar.dma_start(dst[:, :n], ccout[:])

        # ---- u_hat generation: block-diag matmuls, 4 i's per PSUM chunk
        def gen_group(g):
            ps = psgp.tile([128, 1024], F32, tag="psg")
            t, s = divmod(g, 3)
            wt = wbd_tiles[t // WBC]
            wof = (t % WBC) * 640
            for half in range(2):     # j0/j1 cols then j2/j3 cols
                nc.tensor.matmul(
                    ps[:, half * 512 :][:, :320],
                    xt2[s * 32 : (s + 1) * 32, t * 128 : (t + 1) * 128],
                    wt[s * 32 : (s + 1) * 32, wof + half * 320 :][:, :320],
                    start=True,
                    stop=True,
                )
            src = ps[:].rearrange("p (b x) -> p b x", b=2)[:, :, :320]
            dst = uhat[:, g * 640 : (g + 1) * 640].rearrange(
                "p (b x) -> p b x", b=2
            )
            if g < 32 and g % 2 == 0:
                nc.vector.tensor_copy(dst, src)     # startup: DVE has slack
            else:
                nc.scalar.copy(dst, src)

        # early gen groups fill the pre-v0 idle window on PE/DVE
        for g in range(8):
            gen_group(g)

        # ---- round 0: s0 = sum_i softmax_c(bias)[i,c] * u_hat; the softmax
        # weights are folded into w2d on the host (uniform 1/C for zero bias)
        ps0 = ps0p.tile([128, CV], F32, tag="pscom")
        for t in range(NT):
            nc.tensor.matmul(
                ps0[:],
                xt2[:, t * 128 : (t + 1) * 128],
                w2d[:, t * CV : (t + 1) * CV],
                start=(t == 0),
                stop=(t == NT - 1),
            )
        nc.vector.tensor_copy(s_part[:], ps0[:])
        all_reduce(0, s_part, s_tot, F32, CV)
        # bulk u_hat generation: all groups issued up front; DVE takes the
        # even evictions below g=40, Act the rest, streaming through the AR0
        # window and the first half of round 1
        for g in range(8, 31):
            gen_group(g)
        _squash(nc, sp, s_tot[:], v_b)
        for g in range(31, NG):
            gen_group(g)
        prewarm(ACTF.Exp, v_b[:, 0:1])

        # ---- routing rounds 1, 2
        lg1 = None
        for rnd in (1, 2):
            raw = ta if rnd == 1 else tb

            # phase 1: logits chunks l[b,(j,c)] = sum_v u_hat*v  (DVE/Pool)
            for j0, w, on_pool in CHUNK_PLAN:
                eng = nc.gpsimd if on_pool else nc.vector
                ks = slice(j0 * C, (j0 + w) * C)
                uh = uhat[:, j0 * CV : (j0 + w) * CV]
                if on_pool:
                    tmp = gp_scr.tile([128, 16 * CV], BF16, tag="ringg")
                    tre = gp_scr.tile([128, 1600], BF16, tag="treeg")
                else:
                    tmp = rp.tile([128, 16 * CV], BF16, tag="ring")
                    tre = rp.tile([128, 1600], BF16, tag="tree")
                h8 = w * 80            # elems in the v=8 tree level
                vb3 = _ap(v_b[:], [[0, w], [16, C], [1, V]])
                eng.tensor_mul(
                    tmp[:, : w * CV].rearrange("p (i c v) -> p i c v", c=C, v=V),
                    uh.rearrange("p (i c v) -> p i c v", c=C, v=V),
                    vb3,
                )
                t16 = tmp[:, : w * CV].rearrange("p (x v) -> p x v", v=16)
                t8 = tre[:, 0:h8].rearrange("p (x v) -> p x v", v=8)
                t4 = tmp[:, 0 : h8 // 2].rearrange("p (x v) -> p x v", v=4)
                t2 = tre[:, h8 : h8 + h8 // 4].rearrange("p (x v) -> p x v", v=2)
                eng.tensor_add(t8, t16[:, :, 0:8], t16[:, :, 8:16])
                eng.tensor_add(t4, t8[:, :, 0:4], t8[:, :, 4:8])
                eng.tensor_add(t2, t4[:, :, 0:2], t4[:, :, 2:4])
                eng.tensor_add(
                    raw[:, ks],
                    t2[:, :, 0:1].rearrange("p x v -> p (x v)"),
                    t2[:, :, 1:2].rearrange("p x v -> p (x v)"),
                )
                if rnd == 1 and use_bias:
                    eng.tensor_add(raw[:, ks], raw[:, ks], biasr[:, ks])
                if rnd == 2:
                    eng.tensor_add(raw[:, ks], raw[:, ks], lg1[:, ks])
                # exp scatters (j,c) -> (c,j): Act is 1x regardless, free
                raw_k = _ap(raw[:, ks.start :], [[C, w], [1, C]])
                et_k = _ap(et[:, j0:], [[1, w], [IQ, C]])
                nc.scalar.activation(et_k, raw_k, ACTF.Exp)
            # remaining softmax partition sums, deferred past the chunk muls
            zrem = Z_PLAN
            for jz, wz in zrem:
                kz = slice(jz, jz + wz)
                ein = _ap(et[:, jz:], [[1, wz], [IQ, C]])
                nc.vector.tensor_reduce(zsum[:, kz], ein, axis=AX.X, op=ALU.add)
                with nc.allow_low_precision(reason="1/z to bf16: couplings tolerate 0.4% scale noise"):
                    nc.vector.reciprocal(rz[:, kz], zsum[:, kz])

            # softmax tail: cw[b,(c,j)] = et * (1/z) broadcast over c (2x),
            # in 2-c slices so the PE transposes start on the first slice
            for cf in range(5):
                co = cf * 2 * IQ
                rz_b = _ap(rz[:], [[0, 2], [1, IQ]])
                nc.vector.tensor_mul(
                    cw[:, co : co + 2 * IQ].rearrange("p (c j) -> p c j", c=2),
                    et[:, co : co + 2 * IQ].rearrange("p (c j) -> p c j", c=2),
                    rz_b,
                )

            # transpose cw -> cwT[(c,j), b] in 5 psum fills of 6 tiles (2 c's)
            cwT_tiles = []
            for f in range(5):
                pst = psgp.tile([96, 6 * 128], F32, tag="psg")
                cwt = cwp.tile([96, 6 * 128], BF16, tag=f"cwt{f}")
                for t6 in range(6):
                    tix = f * 6 + t6      # global tile (c, jt) index = c*3+jt
                    nc.tensor.matmul(
                        pst[:, t6 * 128 : (t6 + 1) * 128],
                        cw[:, tix * 96 : (tix + 1) * 96],
                        ident[:],
                        start=True,
                        stop=True,
                        tile_position=(0, 0),
                    )
                nc.scalar.copy(cwt[:], pst[:])
                cwT_tiles.append(cwt)

            # ws stage: per c, xcT = cwT*xT (DVE), 24 PE matmuls into psum
            sT = psTp.tile([128, 3 * 128], F32, tag="sT")
            nc.vector.memset(sT[:], 0.0)
            for c in range(C):
                cwt = cwT_tiles[c // 2]
                cof = (c % 2) * JT * 128
                xcq = xp.tile([96, 24 * 128], BF16, tag="xcq")
                in0 = _ap(cwt[:, cof:], [[0, D], [128, JT], [1, 128]])
                in1 = _ap(xt[:], [[JT * 128, D], [128, JT], [1, 128]])
                out = _ap(xcq[:], [[JT * 128, D], [128, JT], [1, 128]])
                xeng = nc.gpsimd if c in POOL_XC else nc.vector
                xeng.tensor_mul(out, in0, in1)
                pb = 32 * (c % 4)
                cb = 128 * (c // 4)
                for dd in range(D):
                    for jt in range(JT):
                        tk = dd * JT + jt
                        nc.tensor.matmul(
                            sT[pb : pb + 16, cb : cb + 128],
                            w2[:, (c * 24 + tk) * 16 : (c * 24 + tk + 1) * 16],
                            xcq[:, tk * 128 : (tk + 1) * 128],
                            start=(tk == 0),
                            stop=(tk == 23),
                            tile_position=(0, pb),
                        )

            # transpose sTe back to batch-partitioned form: 3 full-width
            # PE transposes (lhsT at base 0; offset-sliced lhsT reads crash
            # the HW path), then strided gathers into s_part order
            psr = ps0p.tile([128, 3 * 128], F32, tag="pscom")
            nc.scalar.copy(sTe[:], sT[:])
            for jg in range(3):
                nc.tensor.matmul(
                    psr[:, jg * 128 : (jg + 1) * 128],
                    sTe[:, jg * 128 : (jg + 1) * 128],
                    ident[:],
                    start=True,
                    stop=True,
                    tile_position=(0, 0),
                )
            prewarm(ACTF.Sqrt, sTe[:, 0:1])   # table loads during the AR
            spart = s_part if rnd == 1 else s_part32
            for jg in range(3):
                ncv = 4 if jg < 2 else 2
                dstg = _ap(spart[:, jg * 64 :], [[16, ncv], [1, 16]])
                srcg = _ap(psr[:, jg * 128 :], [[32, ncv], [1, 16]])
                nc.vector.tensor_copy(dstg, srcg)
            all_reduce(rnd, spart, s_tot, F32, CV)
            if rnd == 1:
                _squash(nc, sp, s_tot[:], v_b)
                lg1 = raw
                prewarm(ACTF.Exp, v_b[:, 0:1])
            else:
                _squash(nc, sp, s_tot[:], v_f)

        nc.sync.dma_start(out_d[:], v_f[:])


_PROGRAMS = {}


def _get_program(use_bias=False, cc_stub=False):
    key = (use_bias, cc_stub)
    if key not in _PROGRAMS:
        nc = bacc.Bacc(
            "TRN2", target_bir_lowering=False, debug=False, num_devices=8
        )
        with tile.TileContext(nc) as tc:
            _emit(nc, tc, use_bias, cc_stub)
        nc.compile()
        _PROGRAMS[key] = nc
    return _PROGRAMS[key]


def make_in_maps(inputs, W, bias):
    assert tuple(np.shape(inputs)) == (B, I, D), np.shape(inputs)
    assert tuple(np.shape(W)) == (I, C, D, V), np.shape(W)
    assert tuple(np.shape(bias)) == (1, I, C, 1), np.shape(bias)
    use_bias = bool(np.any(np.asarray(bias)))
    in_maps = []
    for k in range(8):
        bh, iq = k // 4, k % 4
        xs = np.asarray(inputs[bh * NB : (bh + 1) * NB, iq * IQ : (iq + 1) * IQ, :])
        ws = np.asarray(W[iq * IQ : (iq + 1) * IQ])  # [288, 10, 8, 16]

        xT = xs.reshape(NB, IQ * D).T  # [2304, 128] rows (j,d)
        xt2 = xT.reshape(NT, 96, NB).transpose(1, 0, 2).reshape(96, NT * NB)

        Wt = ws.transpose(0, 2, 1, 3)  # [288, 8, 10, 16] (j, d, c, v)
        bs = np.asarray(bias[0, iq * IQ : (iq + 1) * IQ, :, 0], dtype=np.float64)
        eb = np.exp(bs - bs.max(axis=1, keepdims=True))
        cb = (eb / eb.sum(axis=1, keepdims=True)).astype(np.float32)  # [288, 10]
        Wt_s = Wt * cb[:, None, :, None]  # fold round-0 softmax into s0 weights
        w2dense = Wt_s.reshape(IQ * D, CV)  # [(j,d), (c,v)]
        w2d = w2dense.reshape(NT, 96, CV).transpose(1, 0, 2).reshape(96, NT * CV)

        bd = np.zeros((NG, 32, 640), dtype=np.float32)
        Wg = Wt.reshape(NG, 4, D, CV)
        for j in range(4):
            bd[:, j * D : (j + 1) * D, j * CV : (j + 1) * CV] = Wg[:, j]
        wbd = bd.reshape(NT, 96, 640).transpose(1, 0, 2).reshape(96, NT * 640)

        # v2: xT in (d, j) row order, 24 tiles of 96 rows -> [96, 24*128]
        A = xs.transpose(2, 1, 0).reshape(D * IQ, NB)  # row = d*288+j
        xt = A.reshape(24, 96, NB).transpose(1, 0, 2).reshape(96, 24 * 128)

        # v2: ws weights, dense [96, 16] tiles indexed (c*24 + d*3 + jt)
        Wc = ws.transpose(1, 2, 0, 3)  # [c, d, j, v]
        w2t = Wc.reshape(C, D, JT, 96, V).transpose(3, 0, 1, 2, 4)
        w2 = w2t.reshape(96, C * D * JT * V)

        ident = np.eye(128, dtype=np.float32)

        m = {
            "xt2": np.ascontiguousarray(xt2).astype(ml_dtypes.bfloat16),
            "wbd": np.ascontiguousarray(wbd).astype(ml_dtypes.bfloat16),
            "w2d": np.ascontiguousarray(w2d).astype(ml_dtypes.bfloat16),
            "xt": np.ascontiguousarray(xt).astype(ml_dtypes.bfloat16),
            "w2": np.ascontiguousarray(w2).astype(ml_dtypes.bfloat16),
            "ident": ident.astype(ml_dtypes.bfloat16),
            "ident32": ident,
        }
        if use_bias:
            bs = np.asarray(bias[0, iq * IQ : (iq + 1) * IQ, :, 0])
            biasr = np.broadcast_to(bs.reshape(1, IQ * C), (128, IQ * C))
            m["biasr"] = np.ascontiguousarray(biasr).astype(ml_dtypes.bfloat16)
        in_maps.append(m)
    return use_bias, in_maps


def run(inputs, W, bias, **kw):
    use_bias, in_maps = make_in_maps(inputs, W, bias)
    nc = _get_program(use_bias)
    res = run_bass_kernel_spmd(nc, in_maps, core_ids=list(range(8)), **kw)
    outs = res.results
    o0 = np.asarray(outs[0]["out"], dtype=np.float32).reshape(NB, C, V)
    o1 = np.asarray(outs[4]["out"], dtype=np.float32).reshape(NB, C, V)
    return np.concatenate([o0, o1], axis=0), res


def kernel(inputs, W, bias):
    out, _ = run(inputs, W, bias)
    return out


# revision 21
# speedup vs baseline: 1.0093x; 1.0093x over previous
"""CapsuleLayer dynamic-routing kernel for 8 TRN2 NeuronCores.

Problem: inputs [256,1152,8] f32, W [1152,10,8,16] f32, bias [1,1152,10,1] f32.
  u_hat = einsum('bid,icdv->bicv', inputs, W)
  3 rounds of routing (softmax over c, weighted sum over i, squash over v).
Output: [256, 10, 16] f32.

Sharding: 2-way batch x 4-way input-capsule (i) grid over 8 cores.
Core k: batch half k//4 (128 rows), i-quarter k%4 (288 i's).
Per-round partial sums over i are combined with an AllReduce over each
group of 4 cores ([0..3] and [4..7]). Output halves read from cores 0, 4.

v2 design: the weighted-sum stage s = sum_i c*u_hat runs on the PE
instead of DVE/Pool chains:
  - coupling weights cw[b,(c,j)] are transposed by PE identity matmuls
    into cwT[(c,j), b] tiles,
  - xcT[(c,d,j), b] = cwT * xT elementwise (DVE, 2x bf16),
  - s^T[(c,v), b] accumulates in PSUM via dense [96j,16v] W2 matmuls,
  - tiny [16,128] PE transposes bring s back to batch-partitioned form.
Logits l = sum_v u_hat*v remain mul+halving-tree chunks on DVE with some
chunks offloaded to Pool. exp on Act scatters (j,c)->(c,j) so softmax
tail ops stay in DVE 2x mode.
"""

import sys

if "/opt/trn_rl_repo" not in sys.path:
    sys.path.insert(0, "/opt/trn_rl_repo")

import numpy as np
import ml_dtypes

import concourse.bass as bass
from concourse import bacc, mybir, tile
from concourse.bass_utils import run_bass_kernel_spmd

F32 = mybir.dt.float32
BF16 = mybir.dt.bfloat16
AX = mybir.AxisListType
ALU = mybir.AluOpType
ACTF = mybir.ActivationFunctionType

B, I, D, C, V = 256, 1152, 8, 10, 16
CV = C * V                     # 160
NB = 128                       # batch rows per core
IQ = 288                       # i's per core ("j" below)
NG = IQ // 4                   # 72 groups of 4 i's (K=32 block-diag matmuls)
NT = NG // 3                   # 24 tiles of 96 partition-rows
EPS = 1e-7

# logits chunk plan: (j-start, width, on_pool); Pool keeps narrow chunks so
# its scratch stays small, DVE uses wide chunks to amortize per-op overheads
# logits chunks: 18 of width 16; the first four run on Pool
CHUNK_PLAN = [(16 * k, 16, k < 4) for k in range(18)]
Z_PLAN = [(48 * b, 48) for b in range(6)]
JT = 3                         # 96-row j-tiles per (c) or (d)

REPLICA_GROUPS = [[0, 1, 2, 3], [4, 5, 6, 7]]

# logits chunks offloaded to Pool (gpsimd), per round

# xc muls offloaded to Pool in the ws phase (empty: a slow Pool mul stalls
# the in-order PE queue and resets its pstate ramp)
POOL_XC = set()


def _ap(ap, dims):
    """Build an AP with explicit [step, count] free dims (partition dim kept)."""
    return bass.AP(ap.tensor, ap.offset, [list(ap.ap[0])] + [list(d) for d in dims])


def _squash(nc, pool, s_in, v_out):
    """v = (|s|^2/(1+|s|^2)) * s / sqrt(|s|^2 + EPS), norms over v (16).

    s_in: [128, 160] f32-ish SBUF AP. Writes v_out (bf16 for routing rounds,
    f32 for the final output round)."""
    sq = pool.tile([128, CV], F32, tag="sq")
    n2 = pool.tile([128, C], F32, tag="n2")
    n2e = pool.tile([128, C], F32, tag="n2e")
    qs = pool.tile([128, C], F32, tag="qs")
    mm = pool.tile([128, C], F32, tag="mm")
    rm = pool.tile([128, C], F32, tag="rm")
    fc = pool.tile([128, C], F32, tag="fc")
    nc.vector.tensor_mul(sq[:], s_in, s_in)
    nc.vector.tensor_reduce(
        n2[:], sq[:].rearrange("p (c v) -> p c v", v=V), axis=AX.X, op=ALU.add
    )
    # f = n2 / ((1+n2) * sqrt(n2+eps))
    nc.vector.tensor_scalar_add(n2e[:], n2[:], EPS)
    nc.scalar.activation(qs[:], n2e[:], ACTF.Sqrt)
    nc.vector.scalar_tensor_tensor(
        mm[:], n2[:], 1.0, qs[:], op0=ALU.add, op1=ALU.mult
    )
    nc.vector.reciprocal(rm[:], mm[:])
    nc.vector.tensor_mul(fc[:], n2[:], rm[:])
    # v = s * f (broadcast f over v)
    f_b = _ap(fc[:], [[1, C], [0, V]])
    s3 = s_in.rearrange("p (c v) -> p c v", v=V)
    nc.vector.tensor_mul(v_out[:].rearrange("p (c v) -> p c v", v=V), s3, f_b)


def _emit(nc, tc, use_bias, cc_stub=False):
    xt2_d = nc.declare_dram_parameter("xt2", [96, NT * 128], BF16, isOutput=False)
    wbd_d = nc.declare_dram_parameter("wbd", [96, NT * 640], BF16, isOutput=False)
    w2d_d = nc.declare_dram_parameter("w2d", [96, NT * CV], BF16, isOutput=False)
    xt_d = nc.declare_dram_parameter("xt", [96, 24 * 128], BF16, isOutput=False)
    w2_d = nc.declare_dram_parameter("w2", [96, 240 * 16], BF16, isOutput=False)
    id_d = nc.declare_dram_parameter("ident", [128, 128], BF16, isOutput=False)
    id32_d = nc.declare_dram_parameter("ident32", [128, 128], F32, isOutput=False)
    if use_bias:
        bias_d = nc.declare_dram_parameter("biasr", [128, IQ * C], BF16, isOutput=False)
    out_d = nc.declare_dram_parameter("out", [128, CV], F32, isOutput=True)

    WBC = 3                    # NT-tiles per streamed wbd chunk
    NWB = NT // WBC            # 8 chunks

    with (
        tc.tile_pool(name="const", bufs=1) as cp,
        tc.tile_pool(name="wbd_ring", bufs=2) as wp,
        tc.tile_pool(name="small", bufs=1) as sp,
        tc.tile_pool(name="ring", bufs=2) as rp,
        tc.tile_pool(name="gscr", bufs=1) as gp_scr,
        tc.tile_pool(name="xcr", bufs=3) as xp,
        tc.tile_pool(name="cwt", bufs=1) as cwp,
        tc.tile_pool(name="ps0", bufs=1, space="PSUM") as ps0p,
        tc.tile_pool(name="psg", bufs=3, space="PSUM") as psgp,
        tc.tile_pool(name="psT", bufs=1, space="PSUM") as psTp,
        tc.tile_pool(name="dram", bufs=1, space="DRAM") as dp,
    ):
        xt2 = cp.tile([96, NT * 128], BF16, tag="xt2")
        w2d = cp.tile([96, NT * CV], BF16, tag="w2d")
        xt = cp.tile([96, 24 * 128], BF16, tag="xt")
        w2 = cp.tile([96, 240 * 16], BF16, tag="w2")
        ident = cp.tile([128, 128], BF16, tag="ident")
        ident32 = cp.tile([128, 128], F32, tag="ident32")
        uhat = cp.tile([128, IQ * CV], BF16, tag="uhat")

        xsl = slice(0, 6 * 128)
        nc.sync.dma_start(xt2[:, xsl], xt2_d[:, xsl])
        wbd_tiles = []
        for wch in range(NWB):
            wt = wp.tile([96, WBC * 640], BF16, tag="wbd")
            if wch < 2:
                nc.sync.dma_start(
                    wt[:], wbd_d[:, wch * WBC * 640 : (wch + 1) * WBC * 640]
                )
            wbd_tiles.append(wt)
        for xch in range(1, 4):
            xsl = slice(xch * 6 * 128, (xch + 1) * 6 * 128)
            nc.sync.dma_start(xt2[:, xsl], xt2_d[:, xsl])
        nc.sync.dma_start(w2d[:], w2d_d[:])
        for wch in range(2, NWB):
            nc.sync.dma_start(
                wbd_tiles[wch][:],
                wbd_d[:, wch * WBC * 640 : (wch + 1) * WBC * 640],
            )
        nc.sync.dma_start(xt[:], xt_d[:])
        nc.sync.dma_start(w2[:], w2_d[:])
        nc.sync.dma_start(ident[:], id_d[:])
        nc.sync.dma_start(ident32[:], id32_d[:])
        if use_bias:
            biasr = cp.tile([128, IQ * C], BF16, tag="biasr")
            nc.sync.dma_start(biasr[:], bias_d[:])

        # persistent small tiles
        warm = sp.tile([128, 1], F32, tag="warm")
        nc.vector.memset(warm[:], 1.0)

        warm2 = sp.tile([128, 1], F32, tag="warm2")

        def prewarm(func, anchor=None):
            # dummy op so the ACT table set loads off the critical path;
            # anchor ties it to freshly-written data so the tile scheduler
            # cannot hoist it ahead of the ops it must follow
            if anchor is None:
                nc.scalar.activation(warm2[:], warm[:], func)
            else:
                # scale=0 keeps the data dependency but feeds func(0)
                nc.scalar.activation(warm2[:], anchor, func, scale=0.0)

        prewarm(ACTF.Sqrt)
        v_f = sp.tile([128, CV], F32, tag="v_f")
        v_b = sp.tile([128, CV], BF16, tag="v_b")
        s_part = sp.tile([128, CV], F32, tag="s_part")
        sTe = sp.tile([128, 3 * 128], BF16, tag="sTe")
        s_part32 = sp.tile([128, CV], F32, tag="s_part32")
        s_tot = sp.tile([128, CV], F32, tag="s_tot")
        ta = sp.tile([128, IQ * C], BF16, tag="ta")   # raw logits (j,c), rotates
        tb = sp.tile([128, IQ * C], BF16, tag="tb")
        et = sp.tile([128, IQ * C], BF16, tag="et")   # exp(logits), (c,j) order
        cw = sp.tile([128, IQ * C], BF16, tag="cw")   # softmax couplings (c,j)
        zsum = sp.tile([128, IQ], F32, tag="zsum")
        rz = sp.tile([128, IQ], BF16, tag="rz")

        def all_reduce(rnd, src, dst, dt, n):
            ccin = dp.tile([128, n], dt, tag=f"ccin{rnd}")
            ccout = dp.tile([128, n], dt, tag=f"ccout{rnd}")
            nc.scalar.dma_start(ccin[:], src[:, :n])
            if cc_stub:
                nc.scalar.dma_start(ccout[:], ccin[:])
            else:
                nc.gpsimd.collective_compute(
                    "AllReduce",
                    ALU.add,
                    replica_groups=REPLICA_GROUPS,
                    ins=[ccin.opt()],
                    outs=[ccout.opt()],
                )
            nc.scalar.dma_start(dst[:, :n], ccout[:])

        # ---- u_hat generation: block-diag matmuls, 4 i's per PSUM chunk
        def gen_group(g):
            ps = psgp.tile([128, 1024], F32, tag="psg")
            t, s = divmod(g, 3)
            wt = wbd_tiles[t // WBC]
            wof = (t % WBC) * 640
            for half in range(2):     # j0/j1 cols then j2/j3 cols
                nc.tensor.matmul(
                    ps[:, half * 512 :][:, :320],
                    xt2[s * 32 : (s + 1) * 32, t * 128 : (t + 1) * 128],
                    wt[s * 32 : (s + 1) * 32, wof + half * 320 :][:, :320],
                    start=True,
                    stop=True,
                )
            src = ps[:].rearrange("p (b x) -> p b x", b=2)[:, :, :320]
            dst = uhat[:, g * 640 : (g + 1) * 640].rearrange(
                "p (b x) -> p b x", b=2
            )
            if g < 32 and g % 2 == 0:
                nc.vector.tensor_copy(dst, src)     # startup: DVE has slack
            else:
                nc.scalar.copy(dst, src)

        # early gen groups fill the pre-v0 idle window on PE/DVE
        for g in range(8):
            gen_group(g)

        # ---- round 0: s0 = sum_i softmax_c(bias)[i,c] * u_hat; the softmax
        # weights are folded into w2d on the host (uniform 1/C for zero bias)
        ps0 = ps0p.tile([128, CV], F32, tag="pscom")
        for t in range(NT):
            nc.tensor.matmul(
                ps0[:],
                xt2[:, t * 128 : (t + 1) * 128],
                w2d[:, t * CV : (t + 1) * CV],
                start=(t == 0),
                stop=(t == NT - 1),
            )
        nc.vector.tensor_copy(s_part[:], ps0[:])
        all_reduce(0, s_part, s_tot, F32, CV)
        # bulk u_hat generation: all groups issued up front; DVE takes the
        # even evictions below g=40, Act the rest, streaming through the AR0
        # window and the first half of round 1
        for g in range(8, 31):
            gen_group(g)
        _squash(nc, sp, s_tot[:], v_b)
        for g in range(31, NG):
            gen_group(g)
        prewarm(ACTF.Exp, v_b[:, 0:1])

        # ---- routing rounds 1, 2
        lg1 = None
        for rnd in (1, 2):
            raw = ta if rnd == 1 else tb

            # phase 1: logits chunks l[b,(j,c)] = sum_v u_hat*v  (DVE/Pool)
            for j0, w, on_pool in CHUNK_PLAN:
                eng = nc.gpsimd if on_pool else nc.vector
                ks = slice(j0 * C, (j0 + w) * C)
                uh = uhat[:, j0 * CV : (j0 + w) * CV]
                if on_pool:
                    tmp = gp_scr.tile([128, 16 * CV], BF16, tag="ringg")
                    tre = gp_scr.tile([128, 1600], BF16, tag="treeg")
                else:
                    tmp = rp.tile([128, 16 * CV], BF16, tag="ring")
                    tre = rp.tile([128, 1600], BF16, tag="tree")
                h8 = w * 80            # elems in the v=8 tree level
                vb3 = _ap(v_b[:], [[0, w], [16, C], [1, V]])
                eng.tensor_mul(
                    tmp[:, : w * CV].rearrange("p (i c v) -> p i c v", c=C, v=V),
                    uh.rearrange("p (i c v) -> p i c v", c=C, v=V),
                    vb3,
                )
                t16 = tmp[:, : w * CV].rearrange("p (x v) -> p x v", v=16)
                t8 = tre[:, 0:h8].rearrange("p (x v) -> p x v", v=8)
                t4 = tmp[:, 0 : h8 // 2].rearrange("p (x v) -> p x v", v=4)
                t2 = tre[:, h8 : h8 + h8 // 4].rearrange("p (x v) -> p x v", v=2)
                eng.tensor_add(t8, t16[:, :, 0:8], t16[:, :, 8:16])
                eng.tensor_add(t4, t8[:, :, 0:4], t8[:, :, 4:8])
                eng.tensor_add(t2, t4[:, :, 0:2], t4[:, :, 2:4])
                eng.tensor_add(
                    raw[:, ks],
                    t2[:, :, 0:1].rearrange("p x v -> p (x v)"),
                    t2[:, :, 1:2].rearrange("p x v -> p (x v)"),
                )
                if rnd == 1 and use_bias:
                    eng.tensor_add(raw[:, ks], raw[:, ks], biasr[:, ks])
                if rnd == 2:
                    eng.tensor_add(raw[:, ks], raw[:, ks], lg1[:, ks])
                # exp scatters (j,c) -> (c,j), batched per 3 chunks to cut
                # Act per-op overhead (Act co-paces round 1 with evictions)
                if j0 % 48 == 32:
                    jb = j0 - 32
                    raw_k = _ap(raw[:, jb * C :], [[C, 48], [1, C]])
                    et_k = _ap(et[:, jb:], [[1, 48], [IQ, C]])
                    nc.scalar.activation(et_k, raw_k, ACTF.Exp)
            # remaining softmax partition sums, deferred past the chunk muls
            zrem = Z_PLAN
            for jz, wz in zrem:
                kz = slice(jz, jz + wz)
                ein = _ap(et[:, jz:], [[1, wz], [IQ, C]])
                nc.vector.tensor_reduce(zsum[:, kz], ein, axis=AX.X, op=ALU.add)
                with nc.allow_low_precision(reason="1/z to bf16: couplings tolerate 0.4% scale noise"):
                    nc.vector.reciprocal(rz[:, kz], zsum[:, kz])

            # softmax tail: cw[b,(c,j)] = et * (1/z) broadcast over c (2x),
            # in 2-c slices so the PE transposes start on the first slice
            for cf in range(5):
                co = cf * 2 * IQ
                rz_b = _ap(rz[:], [[0, 2], [1, IQ]])
                nc.vector.tensor_mul(
                    cw[:, co : co + 2 * IQ].rearrange("p (c j) -> p c j", c=2),
                    et[:, co : co + 2 * IQ].rearrange("p (c j) -> p c j", c=2),
                    rz_b,
                )

            # transpose cw -> cwT[(c,j), b] in 5 psum fills of 6 tiles (2 c's)
            cwT_tiles = []
            for f in range(5):
                pst = psgp.tile([96, 6 * 128], F32, tag="psg")
                cwt = cwp.tile([96, 6 * 128], BF16, tag=f"cwt{f}")
                for t6 in range(6):
                    tix = f * 6 + t6      # global tile (c, jt) index = c*3+jt
                    nc.tensor.matmul(
                        pst[:, t6 * 128 : (t6 + 1) * 128],
                        cw[:, tix * 96 : (tix + 1) * 96],
                        ident[:],
                        start=True,
                        stop=True,
                        tile_position=(0, 0),
                    )
                nc.scalar.copy(cwt[:], pst[:])
                cwT_tiles.append(cwt)

            # ws stage: per c, xcT = cwT*xT (DVE), 24 PE matmuls into psum
            sT = psTp.tile([128, 3 * 128], F32, tag="sT")
            nc.vector.memset(sT[:], 0.0)
            for c in range(C):
                cwt = cwT_tiles[c // 2]
                cof = (c % 2) * JT * 128
                xcq = xp.tile([96, 24 * 128], BF16, tag="xcq")
                in0 = _ap(cwt[:, cof:], [[0, D], [128, JT], [1, 128]])
                in1 = _ap(xt[:], [[JT * 128, D], [128, JT], [1, 128]])
                out = _ap(xcq[:], [[JT * 128, D], [128, JT], [1, 128]])
                xeng = nc.gpsimd if c in POOL_XC else nc.vector
                xeng.tensor_mul(out, in0, in1)
                pb = 32 * (c % 4)
                cb = 128 * (c // 4)
                for dd in range(D):
                    for jt in range(JT):
                        tk = dd * JT + jt
                        nc.tensor.matmul(
                            sT[pb : pb + 16, cb : cb + 128],
                            w2[:, (c * 24 + tk) * 16 : (c * 24 + tk + 1) * 16],
                            xcq[:, tk * 128 : (tk + 1) * 128],
                            start=(tk == 0),
                            stop=(tk == 23),
                            tile_position=(0, pb),
                        )

            # transpose sTe back to batch-partitioned form: 3 full-width
            # PE transposes (lhsT at base 0; offset-sliced lhsT reads crash
            # the HW path), then strided gathers into s_part order
            psr = ps0p.tile([128, 3 * 128], F32, tag="pscom")
            nc.scalar.copy(sTe[:], sT[:])
            for jg in range(3):
                nc.tensor.matmul(
                    psr[:, jg * 128 : (jg + 1) * 128],
                    sTe[:, jg * 128 : (jg + 1) * 128],
                    ident[:],
                    start=True,
                    stop=True,
                    tile_position=(0, 0),
                )
            prewarm(ACTF.Sqrt, sTe[:, 0:1])   # table loads during the AR
            spart = s_part if rnd == 1 else s_part32
            for jg in range(3):
                ncv = 4 if jg < 2 else 2
                dstg = _ap(spart[:, jg * 64 :], [[16, ncv], [1, 16]])
                srcg = _ap(psr[:, jg * 128 :], [[32, ncv], [1, 16]])
                nc.vector.tensor_copy(dstg, srcg)
            all_reduce(rnd, spart, s_tot, F32, CV)
            if rnd == 1:
                _squash(nc, sp, s_tot[:], v_b)
                lg1 = raw
                prewarm(ACTF.Exp, v_b[:, 0:1])
            else:
                _squash(nc, sp, s_tot[:], v_f)

        nc.sync.dma_start(out_d[:], v_f[:])


_PROGRAMS = {}


def _get_program(use_bias=False, cc_stub=False):
    key = (use_bias, cc_stub)
    if key not in _PROGRAMS:
        nc = bacc.Bacc(
            "TRN2", target_bir_lowering=False, debug=False, num_devices=8
        )
        with tile.TileContext(nc) as tc:
            _emit(nc, tc, use_bias, cc_stub)
        nc.compile()
        _PROGRAMS[key] = nc
    return _PROGRAMS[key]


def make_in_maps(inputs, W, bias):
    assert tuple(np.shape(inputs)) == (B, I, D), np.shape(inputs)
    assert tuple(np.shape(W)) == (I, C, D, V), np.shape(W)
    assert tuple(np.shape(bias)) == (1, I, C, 1), np.shape(bias)
    use_bias = bool(np.any(np.asarray(bias)))
    in_maps = []
    for k in range(8):
        bh, iq = k // 4, k % 4
        xs = np.asarray(inputs[bh * NB : (bh + 1) * NB, iq * IQ : (iq + 1) * IQ, :])
        ws = np.asarray(W[iq * IQ : (iq + 1) * IQ])  # [288, 10, 8, 16]

        xT = xs.reshape(NB, IQ * D).T  # [2304, 128] rows (j,d)
        xt2 = xT.reshape(NT, 96, NB).transpose(1, 0, 2).reshape(96, NT * NB)

        Wt = ws.transpose(0, 2, 1, 3)  # [288, 8, 10, 16] (j, d, c, v)
        bs = np.asarray(bias[0, iq * IQ : (iq + 1) * IQ, :, 0], dtype=np.float64)
        eb = np.exp(bs - bs.max(axis=1, keepdims=True))
        cb = (eb / eb.sum(axis=1, keepdims=True)).astype(np.float32)  # [288, 10]
        Wt_s = Wt * cb[:, None, :, None]  # fold round-0 softmax into s0 weights
        w2dense = Wt_s.reshape(IQ * D, CV)  # [(j,d), (c,v)]
        w2d = w2dense.reshape(NT, 96, CV).transpose(1, 0, 2).reshape(96, NT * CV)

        bd = np.zeros((NG, 32, 640), dtype=np.float32)
        Wg = Wt.reshape(NG, 4, D, CV)
        for j in range(4):
            bd[:, j * D : (j + 1) * D, j * CV : (j + 1) * CV] = Wg[:, j]
        wbd = bd.reshape(NT, 96, 640).transpose(1, 0, 2).reshape(96, NT * 640)

        # v2: xT in (d, j) row order, 24 tiles of 96 rows -> [96, 24*128]
        A = xs.transpose(2, 1, 0).reshape(D * IQ, NB)  # row = d*288+j
        xt = A.reshape(24, 96, NB).transpose(1, 0, 2).reshape(96, 24 * 128)

        # v2: ws weights, dense [96, 16] tiles indexed (c*24 + d*3 + jt)
        Wc = ws.transpose(1, 2, 0, 3)  # [c, d, j, v]
        w2t = Wc.reshape(C, D, JT, 96, V).transpose(3, 0, 1, 2, 4)
        w2 = w2t.reshape(96, C * D * JT * V)

        ident = np.eye(128, dtype=np.float32)

        m = {
            "xt2": np.ascontiguousarray(xt2).astype(ml_dtypes.bfloat16),
            "wbd": np.ascontiguousarray(wbd).astype(ml_dtypes.bfloat16),
            "w2d": np.ascontiguousarray(w2d).astype(ml_dtypes.bfloat16),
            "xt": np.ascontiguousarray(xt).astype(ml_dtypes.bfloat16),
            "w2": np.ascontiguousarray(w2).astype(ml_dtypes.bfloat16),
            "ident": ident.astype(ml_dtypes.bfloat16),
            "ident32": ident,
        }
        if use_bias:
            bs = np.asarray(bias[0, iq * IQ : (iq + 1) * IQ, :, 0])
            biasr = np.broadcast_to(bs.reshape(1, IQ * C), (128, IQ * C))
            m["biasr"] = np.ascontiguousarray(biasr).astype(ml_dtypes.bfloat16)
        in_maps.append(m)
    return use_bias, in_maps


def run(inputs, W, bias, **kw):
    use_bias, in_maps = make_in_maps(inputs, W, bias)
    nc = _get_program(use_bias)
    res = run_bass_kernel_spmd(nc, in_maps, core_ids=list(range(8)), **kw)
    outs = res.results
    o0 = np.asarray(outs[0]["out"], dtype=np.float32).reshape(NB, C, V)
    o1 = np.asarray(outs[4]["out"], dtype=np.float32).reshape(NB, C, V)
    return np.concatenate([o0, o1], axis=0), res


def kernel(inputs, W, bias):
    out, _ = run(inputs, W, bias)
    return out
